# revision 21
# baseline (speedup 1.0000x reference)
"""Self-contained Trainium2 kernel for a dense transformer block.

Contract: kernel(**inputs) takes the FULL fp32 inputs of reference.setup_inputs()
and returns the FULL [2, 2048, 1024] fp32 output, distributing across 8
NeuronCores internally (token-sharded LN/proj/FFN + head-sharded attention,
one AllGather + one AllToAll).

Precision: fp8e4m3 DoubleRow matmuls for qkv / PV / proj / FFN2 with 2-term
(hi + lo/16) fp8 weights; bf16 for scores and FFN1; fp32 residual stream.
"""

import numpy as np
import ml_dtypes

# ---- problem constants (hardcoded per contract) ----
B, T, D = 2, 2048, 1024
NH, DK = 16, 64
DFF = 4096
LN_EPS = 1e-5
NC_ = 8                 # cores
TS = 512                # tokens per core
P = 128                 # partitions
FC = D // P             # 8 feature chunks
M1 = DFF // P           # 32 dff tiles
NQT = 4                 # 512-token q tiles per batch
SCALE = 1.0 / np.sqrt(DK)
SX = 8.0                # fp8 scale for LN1 output

F32 = None
BF16 = None
FP8 = None

_DEF_SCALES = {"dq_q": 1.0, "dq_k": 1.0, "sv_mul": 1.0, "dq_p": 1.0,
               "dq_1": 1.0, "dq_2": 1.0}


def build(nc, tile, mybir, bass, solo=False, scales=None):
    """Emit the SPMD per-core program into `nc` via TileContext."""
    global F32, BF16, FP8
    F32 = mybir.dt.float32
    BF16 = mybir.dt.bfloat16
    FP8 = mybir.dt.float8e4
    sc = scales or _DEF_SCALES
    DR = mybir.MatmulPerfMode.DoubleRow

    # ---- DRAM I/O ----
    x_d = nc.dram_tensor("x_sl", [P, FC, TS], F32, kind="ExternalInput").ap()
    xb_d = nc.dram_tensor("x_bf", [P, FC, TS], BF16, kind="ExternalInput").ap()
    wqk_d = nc.dram_tensor("wqk8", [P, 16, 256], FP8, kind="ExternalInput").ap()
    wv_d = nc.dram_tensor("wv8", [P, 16, 128], FP8, kind="ExternalInput").ap()
    wproj_d = nc.dram_tensor("wproj8", [P, 16, FC, P], FP8, kind="ExternalInput").ap()
    w18_d = nc.dram_tensor("w1_8", [M1 // 2, P, 2, 8, P], FP8, kind="ExternalInput").ap()
    w1_d = nc.dram_tensor("w1", [M1 // 2, P, 2, 4, P], BF16, kind="ExternalInput").ap()
    w2_d = nc.dram_tensor("w2_8", [FC, P, 64, P], FP8, kind="ExternalInput").ap()
    out_d = nc.dram_tensor("out_sl", [FC, P, TS], F32, kind="ExternalOutput").ap()

    Exp = mybir.ActivationFunctionType.Exp
    Gelu = mybir.ActivationFunctionType.Gelu
    Sqrt = mybir.ActivationFunctionType.Sqrt
    Mult = mybir.AluOpType.mult
    Add = mybir.AluOpType.add

    with tile.TileContext(nc) as tc:
        import contextlib
        es = contextlib.ExitStack()
        with es:
            const = es.enter_context(tc.tile_pool(name="const", bufs=1))
            persist = es.enter_context(tc.tile_pool(name="persist", bufs=1))
            dram = es.enter_context(tc.tile_pool(name="dram", bufs=1, space="DRAM"))
            work = es.enter_context(tc.tile_pool(name="work", bufs=1))

            ones_bf = const.tile([P, 1], BF16)
            nc.gpsimd.memset(ones_bf[:], 1.0)

            wqk = const.tile([P, 16, 256], FP8)
            wv = const.tile([P, 16, 128], FP8)
            wproj = const.tile([P, 16, FC, P], FP8)

            # persistent activations
            x_fm = persist.tile([P, FC, TS], F32)       # x^T fp32 (residual)
            r1 = persist.tile([P, FC, TS], F32)         # x + attnproj, fp32

            # collective DRAM buffers
            ag_in = dram.tile([D, TS], FP8, name="ag_in")
            ag_out = dram.tile([NC_, D, TS], FP8, name="ag_out",
                               addr_space="Local" if solo else "Shared")
            a2a_in = [dram.tile([NC_, 64, TS], FP8, name=f"a2a_in{i}")
                      for i in range(2)]
            a2a_out = [dram.tile([NC_, 64, TS], FP8, name=f"a2a_out{i}")
                       for i in range(2)]

            # =============== Stage A: load x, LN1 -> fp8, AllGather =========
            with tc.tile_pool(name="poolA", bufs=1) as poolA, \
                 tc.tile_pool(name="psumA", bufs=2, space="PSUM") as psum:
                x_bf = poolA.tile([P, FC, TS], BF16)
                for i in range(4):
                    nc.sync.dma_start(x_bf[:, 2 * i:2 * i + 2, :],
                                      xb_d[:, 2 * i:2 * i + 2, :])
                # qkv weights early (needed right after AG chunk 0 lands);
                # scalar-engine queue keeps them off the SP/AG critical queue
                nc.sync.dma_start(wqk[:], wqk_d[:])
                nc.sync.dma_start(wv[:], wv_d[:])

                aT = poolA.tile([P, FC, TS], FP8)
                _emit_ln(nc, tc, psum, work, mybir, x_bf, aT, ones_bf, SX)

                HD = D // 2
                for i in range(2):
                    nc.sync.dma_start(
                        ag_in[i * HD:(i + 1) * HD].rearrange(
                            "(fc p) t -> p fc t", p=P),
                        aT[:, 4 * i:4 * i + 4, :])
                if solo:
                    # model progressive AllGather delivery, chunk-major
                    for s in range(NC_):
                        for i in range(2):
                            nc.sync.dma_start(
                                ag_out[s][i * HD:(i + 1) * HD],
                                ag_in[i * HD:(i + 1) * HD])
                else:
                    nc.gpsimd.collective_compute(
                        "AllGather", mybir.AluOpType.bypass,
                        replica_groups=[list(range(NC_))],
                        ins=[ag_in.opt()], outs=[ag_out.opt()],
                    )


            # ======= Stage C+D: qkv interleaved with attention (fp8 DR) =====
            with tc.tile_pool(name="poolC", bufs=1) as poolC, \
                 tc.tile_pool(name="agpool", bufs=4) as agpool:
                qT = poolC.tile([P, NC_, TS], BF16)
                kT = poolC.tile([P, NC_, TS], BF16)
                # v~ layout per head block of 128 cols: col 0 = ones,
                # cols 1:64 = zeros, cols 64:128 = v  (PV psum: row 0 =
                # sum(exp), rows 64:128 = V^T P^T)
                v_sb = poolC.tile([P, 32, 256], FP8)
                nc.gpsimd.memset(v_sb[:, :, 0:1], 1.0)
                nc.gpsimd.memset(v_sb[:, :, 1:64], 0.0)
                nc.gpsimd.memset(v_sb[:, :, 128:129], 1.0)
                nc.gpsimd.memset(v_sb[:, :, 129:192], 0.0)

                psumD_cm = tc.tile_pool(name="psumD", bufs=2, space="PSUM")
                psum = psumD_cm.__enter__()

                def emit_qkv(cb):
                    ag_sb = agpool.tile([P, FC, TS], FP8, tag="ag_sb")
                    nc.sync.dma_start(
                        ag_sb[:],
                        ag_out[cb].rearrange("(fc p) t -> p fc t", p=P))
                    ps_q = psum.tile([P, TS], F32, tag="psqk", bufs=2)
                    for t in range(8):   # 4 hi pairs then 4 lo pairs
                        wsl = 2 * t
                        asl = (2 * t) % 8
                        nc.tensor.matmul(ps_q[:], wqk[:, wsl:wsl + 2, 0:128],
                                         ag_sb[:, asl:asl + 2, :],
                                         start=(t == 0), stop=(t == 7),
                                         perf_mode=DR)
                    nc.vector.tensor_scalar_mul(qT[:, cb, :], ps_q[:], sc["dq_q"])
                    ps_k = psum.tile([P, TS], F32, tag="psqk", bufs=2)
                    for t in range(8):
                        wsl = 2 * t
                        asl = (2 * t) % 8
                        nc.tensor.matmul(ps_k[:], wqk[:, wsl:wsl + 2, 128:256],
                                         ag_sb[:, asl:asl + 2, :],
                                         start=(t == 0), stop=(t == 7),
                                         perf_mode=DR)
                    nc.vector.tensor_scalar_mul(kT[:, cb, :], ps_k[:], sc["dq_k"])
                    for st in range(4):
                        ps_v = psum.tile([P, TS], F32, tag="psqk", bufs=2)
                        for t in range(8):
                            wsl = 2 * t
                            asl = (2 * t) % 8
                            nc.tensor.matmul(
                                ps_v[:, 0:P],
                                ag_sb[:, asl:asl + 2, st * P:(st + 1) * P],
                                wv[:, wsl:wsl + 2, :],
                                start=(t == 0), stop=(t == 7),
                                perf_mode=DR)
                        tt = cb * 4 + st
                        nc.vector.tensor_scalar_mul(
                            v_sb[:, tt, 64:128], ps_v[:, 0:64], sc["sv_mul"])
                        nc.vector.tensor_scalar_mul(
                            v_sb[:, tt, 192:256], ps_v[:, 64:128], sc["sv_mul"])

                oT_h = [poolC.tile([P, NC_, TS], FP8, name=f"oT{h}")
                        for h in range(2)]

                def emit_attn_qtile(h, b, qt):
                    hr = slice(h * 64, (h + 1) * 64)
                    ps_o = psum.tile([P, TS], F32, tag="pso", bufs=2)
                    nkc = 4 * qt + 4
                    npr = nkc // 2
                    for pr in range(npr):
                        # chunk pair (2pr, 2pr+1); pair-aligned causal trim
                        dmin = 2 * pr - 4 * qt          # <0 for off-diag pairs
                        c0 = max(0, 128 * dmin)
                        W = TS - c0
                        ps_s = psum.tile([P, 2, TS], F32, tag="pss", bufs=2)
                        for j in range(2):
                            kc = 2 * pr + j
                            cb_k = 4 * b + kc // 4
                            sl = (kc % 4) * P
                            nc.tensor.matmul(
                                ps_s[:, j, 0:W],
                                kT[hr, cb_k, sl:sl + P],
                                qT[hr, 4 * b + qt, c0:TS],
                                start=True, stop=True)
                        pT = work.tile([P, 2, TS], FP8, tag="pT", bufs=3)
                        nc.scalar.activation(pT[:, :, 0:W], ps_s[:, :, 0:W], Exp)
                        if dmin >= 0:
                            # diagonal pair: zero the masked region of exp
                            nc.gpsimd.affine_select(
                                out=pT[:, 0, 0:128], in_=pT[:, 0, 0:128],
                                compare_op=mybir.AluOpType.is_ge,
                                fill=0.0, base=0,
                                pattern=[[1, 128]], channel_multiplier=-1)
                            nc.gpsimd.memset(pT[:, 1, 0:128], 0.0)
                            nc.gpsimd.affine_select(
                                out=pT[:, 1, 128:256], in_=pT[:, 1, 128:256],
                                compare_op=mybir.AluOpType.is_ge,
                                fill=0.0, base=0,
                                pattern=[[1, 128]], channel_multiplier=-1)
                        nc.tensor.matmul(
                            ps_o[:, c0:TS],
                            v_sb[:, 16 * b + 2 * pr:16 * b + 2 * pr + 2,
                                 h * P:(h + 1) * P],
                            pT[:, :, 0:W],
                            start=(pr == 0), stop=(pr == npr - 1),
                            perf_mode=DR)
                    rec = work.tile([1, TS], F32, tag="rec", bufs=2)
                    nc.vector.reciprocal(rec[:], ps_o[0:1, :])
                    recb = work.tile([P, TS], F32, tag="recb", bufs=2)
                    nc.gpsimd.partition_broadcast(recb[:], rec[:])
                    nc.vector.tensor_mul(
                        oT_h[h][64:128, 4 * b + qt, :],
                        ps_o[64:128, :], recb[64:128, :])
                    s = 4 * b + qt
                    nc.sync.dma_start(
                        a2a_in[h][s].rearrange("p t -> p t"),
                        oT_h[h][64:128, s, :])
                    if solo:
                        # model progressive AllToAll delivery + SBUF land
                        nc.sync.dma_start(a2a_out[h][s], a2a_in[h][s])
                        nc.sync.dma_start(
                            attn_fm[h * 64:(h + 1) * 64, s, :],
                            a2a_out[h][s])

                attn_fm = persist.tile([P, NC_, TS], FP8)
                # attention lags qkv by one chunk so reload DMAs are not
                # head-of-line blocked behind a2a DMAs on the SP queue
                for cb in range(NC_ + 1):
                    if cb < NC_:
                        emit_qkv(cb)
                    if cb >= 1:
                        b, qt = (cb - 1) // 4, (cb - 1) % 4
                        for h in range(2):
                            emit_attn_qtile(h, b, qt)
                    if cb == 3:
                        # WAR-gate on a Pool-queue memset so the 2MB loads
                        # cannot preempt the AllGather window on DMA_ENGINES
                        nc.gpsimd.memset(wproj[0:1, 0:1, 0:1, 0:1], 0.0)
                        nc.sync.dma_start(wproj[:], wproj_d[:])
                    if cb == 5:
                        nc.gpsimd.memset(x_fm[0:1, 0:1, 0:1], 0.0)
                        nc.sync.dma_start(x_fm[:], x_d[:])

                if not solo:
                    for h in range(2):
                        nc.gpsimd.collective_compute(
                            "AllToAll", mybir.AluOpType.bypass,
                            replica_groups=[list(range(NC_))],
                            ins=[a2a_in[h].opt()], outs=[a2a_out[h].opt()],
                        )
                        nc.sync.dma_start(
                            attn_fm[h * 64:(h + 1) * 64, :, :],
                            a2a_out[h][:].rearrange("s p t -> p s t"))

                psumD_cm.__exit__(None, None, None)

            # =============== Stage F: proj (fp8 DR) + residual + LN2 ========
            with tc.tile_pool(name="poolF", bufs=1) as poolF:
                psumF_cm = tc.tile_pool(name="psumF", bufs=2, space="PSUM")
                psum = psumF_cm.__enter__()
                r1_bf = poolF.tile([P, FC, TS], BF16)
                ps_sum = psum.tile([1, TS], F32, tag="st1", bufs=1)
                ps_sq = psum.tile([1, TS], F32, tag="st2", bufs=1)
                for m in range(FC):
                    ps_p = psum.tile([P, TS], F32, tag="psp", bufs=3)
                    for t in range(8):   # 4 hi pairs + 4 lo pairs over s
                        wsl = 2 * t
                        asl = (2 * t) % 8
                        nc.tensor.matmul(ps_p[:], wproj[:, wsl:wsl + 2, m, :],
                                         attn_fm[:, asl:asl + 2, :],
                                         start=(t == 0), stop=(t == 7),
                                         perf_mode=DR)
                    nc.vector.scalar_tensor_tensor(
                        out=r1[:, m, :], in0=ps_p[:], scalar=sc["dq_p"],
                        in1=x_fm[:, m, :], op0=Mult, op1=Add)
                    nc.scalar.copy(r1_bf[:, m, :], r1[:, m, :])
                    # LN2 stats pipelined behind the residual chunks
                    _ln_stats_chunk(nc, work, mybir, ps_sum, ps_sq,
                                    r1_bf[:, m, :], ones_bf, m, "sq2")

                # chunks 0:4 of hhat go to fp8 (scale SX) for the DR half of
                # FFN1; chunks 4:8 stay bf16
                bT8 = poolF.tile([P, 4, TS], FP8)
                bT = poolF.tile([P, FC, TS], BF16)
                (n1b8, n2b8), (n1b1, n2b1) = _ln_finish(
                    nc, work, mybir, ps_sum, ps_sq, [SX, 1.0], "b")
                for fc in range(4):
                    _ln_norm_chunk(nc, work, mybir, r1_bf[:, fc, :],
                                   bT8[:, fc, :], n1b8, n2b8, "lnt")
                for fc in range(4, FC):
                    _ln_norm_chunk(nc, work, mybir, r1_bf[:, fc, :],
                                   bT[:, fc, :], n1b1, n2b1, "lnt")
                psumF_cm.__exit__(None, None, None)

                # =============== Stage G: FFN ===============
                with tc.tile_pool(name="hpool", bufs=1) as hpool, \
                     tc.tile_pool(name="w1pool", bufs=4) as w1pool, \
                     tc.tile_pool(name="w2pool", bufs=3) as w2pool, \
                     tc.tile_pool(name="psumG", bufs=2, space="PSUM") as psumG:
                    hT = hpool.tile([P, M1, TS], FP8)
                    for j in range(M1 // 2):
                        # two m1 tiles share one psum pair + one gelu;
                        # K 0:512 in fp8-DR, K 512:1024 in bf16 (scaled to
                        # match the fp8 psum scale; gelu dequants via scale)
                        w18_t = w1pool.tile([P, 2, 8, P], FP8, tag="w18t")
                        nc.sync.dma_start(w18_t[:], w18_d[j])
                        w1_t = w1pool.tile([P, 2, 4, P], BF16, tag="w1t")
                        nc.sync.dma_start(w1_t[:], w1_d[j])
                        ps_h = psumG.tile([P, 2, TS], F32, tag="psh")
                        for half in range(2):
                            for i in range(4):
                                nc.tensor.matmul(
                                    ps_h[:, half, :],
                                    w18_t[:, half, 2 * i:2 * i + 2, :],
                                    bT8[:, (2 * i) % 4:(2 * i) % 4 + 2, :],
                                    start=(i == 0), stop=False,
                                    perf_mode=DR)
                            for fc in range(4):
                                nc.tensor.matmul(
                                    ps_h[:, half, :], w1_t[:, half, fc, :],
                                    bT[:, 4 + fc, :],
                                    start=False, stop=(fc == 3))
                        nc.scalar.activation(
                            hT[:, 2 * j:2 * j + 2, :], ps_h[:], Gelu,
                            scale=sc["dq_1"])

                    for m2 in range(FC):
                        w2_t = w2pool.tile([P, 64, P], FP8, tag="w2t")
                        nc.sync.dma_start(w2_t[:], w2_d[m2])
                        ps_f = psumG.tile([P, TS], F32, tag="psf")
                        for t in range(32):   # 16 hi pairs + 16 lo pairs
                            wsl = 2 * t
                            asl = (2 * t) % 32
                            nc.tensor.matmul(ps_f[:], w2_t[:, wsl:wsl + 2, :],
                                             hT[:, asl:asl + 2, :],
                                             start=(t == 0), stop=(t == 31),
                                             perf_mode=DR)
                        of = work.tile([P, TS], F32, tag="of", bufs=2)
                        nc.vector.scalar_tensor_tensor(
                            out=of[:], in0=ps_f[:], scalar=sc["dq_2"],
                            in1=r1[:, m2, :], op0=Mult, op1=Add)
                        nc.sync.dma_start(out_d[m2], of[:])
    return nc


def _ln_stats_chunk(nc, work, mybir, ps_sum, ps_sq, x_chunk, ones_bf, fc, tag):
    BF16 = mybir.dt.bfloat16
    sq = work.tile([P, TS], BF16, tag=tag, bufs=2)
    nc.vector.tensor_mul(sq[:], x_chunk, x_chunk)
    nc.tensor.matmul(ps_sum[:], ones_bf[:], x_chunk,
                     start=(fc == 0), stop=(fc == FC - 1))
    nc.tensor.matmul(ps_sq[:], ones_bf[:], sq[:],
                     start=(fc == 0), stop=(fc == FC - 1))


def _ln_finish(nc, work, mybir, ps_sum, ps_sq, out_scales, tagp):
    """Compute per-token (n1, n2) broadcast pairs, one per requested
    out_scale: out = x*n1b + n2b = out_scale * (x-mu)/sd."""
    F32 = mybir.dt.float32
    BF16 = mybir.dt.bfloat16
    Sqrt = mybir.ActivationFunctionType.Sqrt
    eps_t = work.tile([1, 1], F32, tag=tagp + "eps")
    nc.gpsimd.memset(eps_t[:], LN_EPS)
    mu = work.tile([1, TS], F32, tag=tagp + "mu")
    nc.scalar.mul(mu[:], ps_sum[:], 1.0 / D)
    msq = work.tile([1, TS], F32, tag=tagp + "msq")
    nc.scalar.mul(msq[:], ps_sq[:], 1.0 / D)
    mu2 = work.tile([1, TS], F32, tag=tagp + "mu2")
    nc.vector.tensor_mul(mu2[:], mu[:], mu[:])
    var = work.tile([1, TS], F32, tag=tagp + "var")
    nc.vector.tensor_sub(var[:], msq[:], mu2[:])
    sd = work.tile([1, TS], F32, tag=tagp + "sd")
    nc.scalar.activation(sd[:], var[:], Sqrt, bias=eps_t[:])
    n1r = work.tile([1, TS], F32, tag=tagp + "n1r")
    nc.vector.reciprocal(n1r[:], sd[:])
    pairs = []
    for i, out_scale in enumerate(out_scales):
        n1 = work.tile([1, TS], BF16, tag=f"{tagp}n1s{i}")
        nc.vector.tensor_scalar_mul(n1[:], n1r[:], float(out_scale))
        n2 = work.tile([1, TS], BF16, tag=f"{tagp}n2{i}")
        nc.vector.scalar_tensor_tensor(
            out=n2[:], in0=mu[:], scalar=-float(out_scale), in1=n1r[:],
            op0=mybir.AluOpType.mult, op1=mybir.AluOpType.mult)
        n1b = work.tile([P, TS], BF16, tag=f"{tagp}n1b{i}")
        nc.gpsimd.partition_broadcast(n1b[:], n1[:])
        n2b = work.tile([P, TS], BF16, tag=f"{tagp}n2b{i}")
        nc.gpsimd.partition_broadcast(n2b[:], n2[:])
        pairs.append((n1b, n2b))
    return pairs


def _ln_norm_chunk(nc, work, mybir, x_chunk, out_chunk, n1b, n2b, tag):
    BF16 = mybir.dt.bfloat16
    t = work.tile([P, TS], BF16, tag=tag, bufs=2)
    nc.vector.tensor_mul(t[:], x_chunk, n1b[:])
    nc.vector.tensor_add(out_chunk, t[:], n2b[:])


def _emit_ln(nc, tc, psum, work, mybir, x_bf, out_t, ones_bf, out_scale):
    F32 = mybir.dt.float32
    ps_sum = psum.tile([1, TS], F32, tag="st1", bufs=1)
    ps_sq = psum.tile([1, TS], F32, tag="st2", bufs=1)
    for fc in range(FC):
        _ln_stats_chunk(nc, work, mybir, ps_sum, ps_sq, x_bf[:, fc, :],
                        ones_bf, fc, "sq")
    ((n1b, n2b),) = _ln_finish(nc, work, mybir, ps_sum, ps_sq, [out_scale], "a")
    for fc in range(FC):
        _ln_norm_chunk(nc, work, mybir, x_bf[:, fc, :], out_t[:, fc, :],
                       n1b, n2b, "lnt")


# ==================== host side ====================

_CACHE = {}
_FP8N = ml_dtypes.float8_e4m3


def _pow2scale(w):
    return 2.0 ** np.floor(np.log2(120.0 / np.abs(w).max()))


def _q8pair(w):
    """2-term fp8 quantization: w*s ~= hi + lo_st*16/16 ... returns
    (hi, lo_st, s) with lo_st pre-divided by 16 (exact pow2 shift) so both
    terms accumulate in one PSUM group at combined scale s."""
    s = _pow2scale(w)
    ws = (w * s).astype(np.float32)
    hi = ws.astype(_FP8N)
    r = (ws - hi.astype(np.float32)) * 16.0
    lo = r.astype(_FP8N)
    lo_st = (lo.astype(np.float32) / 16.0).astype(_FP8N)
    return hi, lo_st, s


def _build_and_compile(scales):
    if "nc" in _CACHE:
        return _CACHE["nc"]
    import concourse.bass as bass
    import concourse.mybir as mybir
    import concourse.tile as tile
    from concourse import bacc
    nc = bacc.Bacc("TRN2", target_bir_lowering=False, debug=False,
                   num_devices=NC_)
    build(nc, tile, mybir, bass, solo=False, scales=scales)
    nc.compile()
    _CACHE["nc"] = nc
    return nc


def _prep_inputs(x, w_qkv, w_proj, w1, w2, ln1_g, ln1_b, ln2_g, ln2_b):
    bf = ml_dtypes.bfloat16
    x = np.asarray(x, np.float32)
    w_qkv = np.asarray(w_qkv, np.float32)
    w_proj = np.asarray(w_proj, np.float32)
    w1 = np.asarray(w1, np.float32)
    w2 = np.asarray(w2, np.float32)
    ln1_g = np.asarray(ln1_g, np.float32)
    ln2_g = np.asarray(ln2_g, np.float32)
    assert not np.any(np.asarray(ln1_b)) and not np.any(np.asarray(ln2_b)), \
        "nonzero LN bias not wired in this build"

    x_flat = np.ascontiguousarray(x.reshape(B * T, D))
    wq = w_qkv[:, :D] * (SCALE * ln1_g[:, None])
    wk = w_qkv[:, D:2 * D] * ln1_g[:, None]
    wv_full = w_qkv[:, 2 * D:] * ln1_g[:, None]
    w1f = w1 * ln2_g[:, None]

    wq_hi, wq_lo, s_wq = _q8pair(wq)
    wk_hi, wk_lo, s_wk = _q8pair(wk)
    wv_hi, wv_lo, s_wv = _q8pair(wv_full)
    wp_hi, wp_lo, s_wp = _q8pair(w_proj)
    w2_hi, w2_lo, s_w2 = _q8pair(w2)
    w18_hi, w18_lo, s_w1 = _q8pair(w1f[:D // 2])

    scales = {
        "dq_q": float(1.0 / (SX * s_wq)),
        "dq_k": float(1.0 / (SX * s_wk)),
        "sv_mul": float(16.0 / (SX * s_wv)),
        "dq_p": float(1.0 / (16.0 * s_wp)),
        "dq_1": float(1.0 / (SX * s_w1)),
        "dq_2": float(1.0 / s_w2),
    }

    # w1 fp8 half: [M1//2, P, 2, 8, P]; slots 0-3 = hi chunks 0-3 of K 0:512,
    # slots 4-7 = lo; w[128*s + p, 128*(2j+half) + c]
    def w1_8pack(wterm):
        return wterm.reshape(4, P, M1 // 2, 2, P).transpose(2, 1, 3, 0, 4)
    w18_t = np.ascontiguousarray(
        np.concatenate([w1_8pack(w18_hi), w1_8pack(w18_lo)], axis=3))
    # w1 bf16 half (K 512:1024), pre-scaled to the fp8 psum scale SX*s_w1
    w1_t = np.ascontiguousarray(
        (w1f[D // 2:] * (SX * s_w1)).reshape(4, P, M1 // 2, 2, P)
        .transpose(2, 1, 3, 0, 4)).astype(bf)
    # w2: [FC, P, 64, P]; kc<32 hi, kc>=32 lo; w2[128*kc + p, 128*m2 + c]
    w2h = w2_hi.reshape(M1, P, FC, P).transpose(2, 1, 0, 3)   # [FC, P, 32, P]
    w2l = w2_lo.reshape(M1, P, FC, P).transpose(2, 1, 0, 3)
    w2_t = np.ascontiguousarray(np.concatenate([w2h, w2l], axis=2))
    # wproj: [P, 16, FC, P]; s<8 hi, s>=8 lo; w[128*s + p, 128*m + c]
    wph = wp_hi.reshape(FC, P, FC, P).transpose(1, 0, 2, 3)   # [P, 8, FC, P]
    wpl = wp_lo.reshape(FC, P, FC, P).transpose(1, 0, 2, 3)
    wproj_t = np.ascontiguousarray(np.concatenate([wph, wpl], axis=1))

    in_maps = []
    for c in range(NC_):
        hcols = slice(2 * c * DK, 2 * c * DK + 128)
        # wqk8 [P, 16, 256]: chunk<8 hi, >=8 lo; cols 0:128 q, 128:256 k
        def qk_pack(qh, kh):
            m = np.concatenate([qh[:, hcols], kh[:, hcols]], axis=1)  # [1024,256]
            return m.reshape(FC, P, 256).transpose(1, 0, 2)           # [P,8,256]
        wqk_t = np.ascontiguousarray(np.concatenate(
            [qk_pack(wq_hi, wk_hi), qk_pack(wq_lo, wk_lo)], axis=1))
        wv_t = np.ascontiguousarray(np.concatenate(
            [wv_hi[:, hcols].reshape(FC, P, P).transpose(1, 0, 2),
             wv_lo[:, hcols].reshape(FC, P, P).transpose(1, 0, 2)], axis=1))
        x_c = x_flat[c * TS:(c + 1) * TS]          # [TS, D]
        x_cT = np.ascontiguousarray(
            x_c.T.reshape(FC, P, TS).transpose(1, 0, 2))  # [P, FC, TS]
        in_maps.append({
            "x_sl": x_cT,
            "x_bf": x_cT.astype(bf),
            "wqk8": wqk_t,
            "wv8": wv_t,
            "wproj8": wproj_t,
            "w1_8": w18_t,
            "w1": w1_t,
            "w2_8": w2_t,
        })
    return in_maps, scales


def kernel(x, w_qkv, w_proj, w1, w2, ln1_g, ln1_b, ln2_g, ln2_b):
    from concourse.bass_utils import run_bass_kernel_spmd
    in_maps, scales = _prep_inputs(x, w_qkv, w_proj, w1, w2,
                                   ln1_g, ln1_b, ln2_g, ln2_b)
    nc = _build_and_compile(scales)
    res = run_bass_kernel_spmd(nc, in_maps, list(range(NC_)))
    slices = []
    for c in range(NC_):
        o = res.results[c]["out_sl"]            # [FC, P, TS]
        slices.append(o.transpose(2, 0, 1).reshape(TS, D))
    out = np.concatenate(slices, axis=0)
    return np.ascontiguousarray(out.reshape(B, T, D)).astype(np.float32)
